# revision 29
# baseline (speedup 1.0000x reference)
"""Trainium2 Bass kernel for nn_AMPBlock0 (BigVGAN AMP block):
x -> SnakeBeta-Activation1d -> Conv1d(512,512,k3) -> SnakeBeta-Activation1d
  -> Conv1d(512,512,k3) -> + x

Distribution: pure data-parallel over batch B=8 across the 8 NeuronCores
(one sample per core, zero collectives).

On-chip math (per activation stage, z [512, T] channels-on-partitions):
  The reference Activation1d (2x linear upsample -> snakebeta -> 2x avgpool)
  collapses algebraically to, per output col t:
    out = 0.5*(u1 + u2) + rb - 0.5*rb*(cos(2a*u1) + cos(2a*u2))
    u1 = 0.75 z[t] + 0.25 z[t-1],  u2 = 0.75 z[t] + 0.25 z[t+1]   (edge-replicate)
  With the product identity cos p + cos q = 2 cos((p+q)/2) cos((p-q)/2) and
  t6 = z/6:
    sv = z + t6[t-1] + t6[t+1]        ( = (u1+u2)/1.5 )
    d6 = t6[t-1] - t6[t+1]            ( a(u1-u2) = 1.5a*d6, always tiny )
    cA = cos(1.5a*sv) via range-reduced sin (arg wrapped into [-pi,pi],
         wrap k computed with a round-to-int16 cast — HW Sin is only
         accurate on |arg|<~3.2)
    cB = cos(1.5a*d6) via plain sin (args stay small)
    out_h = sv - rb'*cA*cB,  rb' = (4/3)/(2 exp(beta) + eps)
  true act A = 0.75*(out_h + rb'); the 0.75 folds into conv weights and the
  +rb' constant folds into the conv bias; conv zero-padding is encoded by
  sentinel cols out_h = -rb'.

Everything is fused per 1016-column chunk: act1 (DVE+ACT) -> conv1 (bf16
TensorE matmuls accumulating over 4 ci-chunks x 3 taps into PSUM) -> bias
eviction (ACT) -> act2 -> conv2 -> residual-add eviction (DVE STT) -> DMA.
"""
import sys
if '/opt/trn_rl_repo' not in sys.path:
    sys.path.insert(0, '/opt/trn_rl_repo')

import numpy as np
import ml_dtypes

import concourse.bass as bass
import concourse.mybir as mybir
import concourse.tile as tile
from concourse import bacc
from concourse.bass_utils import run_bass_kernel_spmd

F32 = mybir.dt.float32
BF16 = mybir.dt.bfloat16
I16 = mybir.dt.int16
AOP = mybir.AluOpType
AF = mybir.ActivationFunctionType

C = 512
T = 8192
NCC = 4          # channel chunks of 128
P = 128
S_CHUNK = 1016   # output cols per chunk
PAD = 4          # host-side x padding cols per side
EPS = 1e-9
TWOPI = 2.0 * np.pi

_NC_CACHE = {}


S_FIRST = 252    # small first chunk: cuts pipeline-fill latency


def _chunks():
    out = [(0, S_FIRST)]
    o0 = S_FIRST
    while o0 < T:
        out.append((o0, min(S_CHUNK, T - o0)))
        o0 += S_CHUNK
    return out


def _segments(width, cap=510):
    segs = []
    c = 0
    while c < width:
        w = min(cap, width - c)
        segs.append((c, w))
        c += w
    return segs


def build_nc():
    nc = bacc.Bacc(None, num_swdge_queues=4)
    NW = 2 * 3 * NCC * NCC
    xp = nc.declare_dram_parameter("xp", [C, T + 2 * PAD], F32, isOutput=False)
    # x + bias2_eff, pre-added on host: the conv2 eviction becomes a single
    # tensor_tensor add (psum + xr2) with no per-partition scalar needed.
    xr2 = nc.declare_dram_parameter("xr2", [C, T], F32, isOutput=False)
    # weights pre-packed to the exact SBUF image: row p = partition p (= ci
    # within a 128-chunk), cols = NW tiles of 128 (co) -> one large DMA with
    # 24.6KB contiguous per partition instead of 12288 256B packets.
    wts = nc.declare_dram_parameter("wts", [P, NW * P], BF16, isOutput=False)
    # params packed [128, 4cc * 12]
    prm = nc.declare_dram_parameter("prm", [P, NCC * 12], F32, isOutput=False)
    outd = nc.declare_dram_parameter("out", [C, T], F32, isOutput=True)

    # param columns per cc (12 each): per act i: 6*i+0 skA, 6*i+1 mmul,
    # 6*i+2 s15a (cA scale), 6*i+3 nrbp (-rb' = sentinel), 6*i+4 beff,
    # 6*i+5 s025a (cB scale = 0.25a)
    with tile.TileContext(nc) as tc:
        with tc.tile_pool(name="wp", bufs=1) as wp, \
             tc.tile_pool(name="pp", bufs=1) as ppool, \
             tc.tile_pool(name="xbp", bufs=8) as xbp, \
             tc.tile_pool(name="tp", bufs=4) as tp, \
             tc.tile_pool(name="zp", bufs=8) as zp, \
             tc.tile_pool(name="iop", bufs=6) as iop, \
             tc.tile_pool(name="ps", bufs=4, space="PSUM") as psp:

            # --- weights: one big SBUF tile, one large DMA ---
            w_all = wp.tile([P, NW * P], BF16)
            nc.sync.dma_start(out=w_all[:], in_=wts[:])

            def wview(conv, dk, ci, co):
                idx = ((conv * 3 + dk) * NCC + ci) * NCC + co
                return w_all[:, idx * P:(idx + 1) * P]

            # --- params (packed [128, 4cc*12]) ---
            prmt = ppool.tile([P, NCC * 12], F32, tag="prm")
            nc.sync.dma_start(out=prmt[:], in_=prm[:])
            pb = ppool.tile([P, 1], F32, tag="pb")
            nc.vector.memset(pb[:], float(np.pi / 2))

            def pcol(cc, j):
                return prmt[:, cc * 12 + j:cc * 12 + j + 1]

            chunks = _chunks()
            last_i = len(chunks) - 1

            def act_stage(src, src_w, dst_w, dst_tag, cc, act):
                """src: SBUF tile [P, src_w] bf16 holding z over cols
                [c0-1, c0-1+src_w) where the dst covers [c0, c0+dst_w).
                Emits the activation pipeline, returns dst tile [P, dst_w]."""
                skA, mmul, s15a, nrbp, _, s025a = (
                    pcol(cc, 6 * act + j) for j in range(6))
                # e6 = z[-1]+z[+1], d6 = z[-1]-z[+1] (unscaled; /6 folded into
                # the sv STT and the cB sin scale 0.25a)
                e6 = tp.tile([P, dst_w], BF16, tag="e6")
                nc.vector.tensor_add(e6[:], src[:, 0:dst_w], src[:, 2:dst_w + 2])
                d6 = tp.tile([P, dst_w], BF16, tag="d6")
                nc.vector.tensor_tensor(d6[:], src[:, 0:dst_w],
                                        src[:, 2:dst_w + 2], AOP.subtract)
                sv = tp.tile([P, dst_w], BF16, tag="sv")
                nc.vector.scalar_tensor_tensor(sv[:], e6[:], 1.0 / 6.0,
                                               src[:, 1:dst_w + 1],
                                               AOP.mult, AOP.add)
                kk = tp.tile([P, dst_w], I16, tag="kk")
                nc.vector.tensor_scalar(kk[:], sv[:], skA, 0.25, AOP.mult, AOP.add)
                mm = tp.tile([P, dst_w], F32, tag="mm")
                nc.vector.scalar_tensor_tensor(mm[:], kk[:], mmul, sv[:],
                                               AOP.mult, AOP.add)
                cA = tp.tile([P, dst_w], BF16, tag="cA")
                nc.scalar.activation(cA[:], mm[:], AF.Sin, bias=pb[:], scale=s15a)
                cB = tp.tile([P, dst_w], BF16, tag="cB")
                nc.scalar.activation(cB[:], d6[:], AF.Sin, bias=pb[:], scale=s025a)
                sc = tp.tile([P, dst_w], BF16, tag="sc")
                nc.vector.tensor_mul(sc[:], cA[:], cB[:])
                gg = tp.tile([P, dst_w], BF16, tag="gg")
                nc.scalar.activation(gg[:], sc[:], AF.Copy, scale=nrbp)
                dst = zp.tile([P, dst_w], BF16, tag=dst_tag)
                nc.vector.tensor_add(dst[:], sv[:], gg[:])
                return dst

            for ci_chunk, (o0, S) in enumerate(chunks):
                first = ci_chunk == 0
                last = ci_chunk == last_i
                Ex = S + 8          # x cols [o0-4, o0+S+4)
                E1 = S + 6          # z1 cols [o0-3, o0+S+3)
                E2 = S + 4          # z2 / conv1-out cols [o0-2, o0+S+2)
                E3 = S + 2          # z3 cols [o0-1, o0+S+1)

                # ---- act1 ----
                z1s = []
                for cc in range(NCC):
                    xb = xbp.tile([P, Ex], BF16, tag="xb")
                    nc.gpsimd.dma_start(out=xb[:],
                                        in_=xp[cc * P:(cc + 1) * P, o0:o0 + Ex])
                    z1 = act_stage(xb, Ex, E1, "z1", cc, 0)
                    if first:
                        # col -1 (local 2) := sentinel1 (conv1 zero-pad)
                        nc.scalar.activation(z1[:, 2:3], pcol(cc, 3), AF.Copy)
                    if last:
                        # col T (local T-(o0-3)=S+3) := sentinel1
                        nc.scalar.activation(z1[:, S + 3:S + 4], pcol(cc, 3),
                                             AF.Copy)
                    z1s.append(z1)

                # ---- conv1 + evict to z2 ----
                z2s = []
                for cc in range(NCC):
                    z2t = zp.tile([P, E2], BF16, tag="z2")
                    z2s.append(z2t)
                segs1 = _segments(E2, 510)
                for co in range(NCC):
                    pss = []
                    for si, (c0, w) in enumerate(segs1):
                        pst = psp.tile([P, w], F32, tag="c1ps")
                        pss.append(pst)
                    n = 0
                    for ci in range(NCC):
                        for dk in range(3):
                            for si, (c0, w) in enumerate(segs1):
                                nc.tensor.matmul(
                                    pss[si][:, :w],
                                    wview(0, dk, ci, co),
                                    z1s[ci][:, c0 + dk:c0 + dk + w],
                                    start=(n == 0), stop=(n == 11))
                            n += 1
                    for si, (c0, w) in enumerate(segs1):
                        nc.scalar.activation(z2s[co][:, c0:c0 + w],
                                             pss[si][:, :w],
                                             AF.Identity, bias=pcol(co, 4),
                                             scale=1.0)
                for cc in range(NCC):
                    z2 = z2s[cc]
                    if first:
                        # cols -2,-1 := replicate y1[0] (local 2)
                        nc.scalar.activation(z2[:, 0:1], z2[:, 2:3], AF.Copy)
                        nc.scalar.activation(z2[:, 1:2], z2[:, 2:3], AF.Copy)
                    if last:
                        # cols T,T+1 (locals S+2,S+3) := replicate y1[T-1] (S+1)
                        nc.scalar.activation(z2[:, S + 2:S + 3],
                                             z2[:, S + 1:S + 2], AF.Copy)
                        nc.scalar.activation(z2[:, S + 3:S + 4],
                                             z2[:, S + 1:S + 2], AF.Copy)

                # ---- act2 ----
                z3s = []
                for cc in range(NCC):
                    z3 = act_stage(z2s[cc], E2, E3, "z3", cc, 1)
                    if first:
                        nc.scalar.activation(z3[:, 0:1], pcol(cc, 9), AF.Copy)
                    if last:
                        nc.scalar.activation(z3[:, S + 1:S + 2], pcol(cc, 9),
                                             AF.Copy)
                    z3s.append(z3)

                # ---- conv2 + residual evict + out DMA ----
                segs2 = _segments(S, 508)
                for co in range(NCC):
                    pss = []
                    xrs = []
                    for si, (c0, w) in enumerate(segs2):
                        pst = psp.tile([P, w], F32, tag="c2ps")
                        pss.append(pst)
                        xr = iop.tile([P, w], F32, tag="xr")
                        nc.sync.dma_start(
                            out=xr[:],
                            in_=xr2[co * P:(co + 1) * P, o0 + c0:o0 + c0 + w])
                        xrs.append(xr)
                    n = 0
                    for ci in range(NCC):
                        for dk in range(3):
                            for si, (c0, w) in enumerate(segs2):
                                nc.tensor.matmul(
                                    pss[si][:, :w],
                                    wview(1, dk, ci, co),
                                    z3s[ci][:, c0 + dk:c0 + dk + w],
                                    start=(n == 0), stop=(n == 11))
                            n += 1
                    for si, (c0, w) in enumerate(segs2):
                        of = iop.tile([P, w], F32, tag="of")
                        nc.vector.tensor_add(of[:], pss[si][:, :w], xrs[si][:])
                        nc.sync.dma_start(
                            out=outd[co * P:(co + 1) * P, o0 + c0:o0 + c0 + w],
                            in_=of[:])
    nc.compile()
    return nc


def _host_prep(x, v1, g1, bias1, v2, g2, bias2, alpha1, beta1, alpha2, beta2):
    def wn(v, g):
        nrm = np.sqrt((v * v).sum(axis=(1, 2), keepdims=True))
        return (g[:, None, None] * v / nrm).astype(np.float32)

    def bf(a):
        return a.astype(ml_dtypes.bfloat16)

    NW = 2 * 3 * NCC * NCC
    prm_c = np.zeros((C, 12), dtype=np.float32)
    wflat = np.zeros((P, NW * P), dtype=ml_dtypes.bfloat16)
    beff2 = None
    for i, (al, be, v, g, b) in enumerate(
            [(alpha1, beta1, v1, g1, bias1), (alpha2, beta2, v2, g2, bias2)]):
        a = np.exp(al).astype(np.float32)
        rbp = ((4.0 / 3.0) / (2.0 * np.exp(be) + EPS)).astype(np.float32)
        W = wn(v, g) * np.float32(0.75)
        Wb = bf(W).astype(np.float32)
        prm_c[:, 6 * i + 0] = 1.5 * a / TWOPI
        prm_c[:, 6 * i + 1] = -TWOPI / (1.5 * a)
        prm_c[:, 6 * i + 2] = 1.5 * a
        prm_c[:, 6 * i + 3] = -rbp
        prm_c[:, 6 * i + 4] = b + np.einsum('oik,i->o', Wb, rbp)
        prm_c[:, 6 * i + 5] = 0.25 * a
        if i == 1:
            beff2 = prm_c[:, 6 * i + 4].copy()
        for dk in range(3):
            for ci in range(NCC):
                for co in range(NCC):
                    idx = ((i * 3 + dk) * NCC + ci) * NCC + co
                    # SBUF image: row p = ci within chunk, col = co
                    blk = W[co * P:(co + 1) * P, ci * P:(ci + 1) * P, dk].T
                    wflat[:, idx * P:(idx + 1) * P] = bf(blk)

    # prm packed [128, 4cc*12]: row p, col cc*12+j = prm_c[cc*128+p, j]
    prm = np.ascontiguousarray(
        prm_c.reshape(NCC, P, 12).transpose(1, 0, 2).reshape(P, NCC * 12))
    xpad = np.pad(x, ((0, 0), (0, 0), (PAD, PAD)), mode='edge').astype(np.float32)
    xr2 = (x + beff2[None, :, None]).astype(np.float32)
    return xpad, xr2, wflat, prm


def kernel(x, v1, g1, bias1, v2, g2, bias2, alpha1, beta1, alpha2, beta2,
           _profile=False):
    x = np.ascontiguousarray(x, dtype=np.float32)
    xpad, xr2, wflat, prm = _host_prep(x, v1, g1, bias1, v2, g2, bias2,
                                       alpha1, beta1, alpha2, beta2)
    if 'nc' not in _NC_CACHE:
        _NC_CACHE['nc'] = build_nc()
    nc = _NC_CACHE['nc']
    B = x.shape[0]
    assert B == 8, f"expected B=8, got {B}"
    in_maps = [{"xp": np.ascontiguousarray(xpad[b]),
                "xr2": np.ascontiguousarray(xr2[b]),
                "wts": wflat, "prm": prm} for b in range(B)]
    res = run_bass_kernel_spmd(nc, in_maps, list(range(8)), trace=_profile)
    out = np.stack([res.results[b]["out"] for b in range(B)])
    if _profile:
        kernel.last_results = res
    return out
